# revision 13
# baseline (speedup 1.0000x reference)
"""Masked self-attention (mask is a no-op) on 8 Trainium2 NeuronCores — v2.

Math (reference):
    q = x @ wq.T ; k = x @ wk.T ; v = x @ wv.T
    O = softmax(q @ k.T / sqrt(D)) @ v

Factorized form (identical math up to fp reassociation):
    W_qk = wq.T @ wk                  # [D, D] weights-only, folded on host
    S    = (x_blk @ W_qk) @ x.T       # block of q @ k.T (unscaled)
    P    = exp(S / sqrt(D))           # unnormalized softmax
    O    = (P @ x) @ wv.T / rowsum(P)

Changes vs v1 (1216.7us):
  * rowsum off the PE: DVE accumulates sum_t PT[t,:] into fp32 rsacc;
    a single ones-matmul pair at the end replaces 128 rowsum matmuls
    (-64K PE cycles = -27us)
  * S/Z PSUM tiles are [P,512] halves with bufs=3: exp/copy never blocks
    the next matmul group (kills the ~200-700ns per-tile PE gaps)
  * two HWDGE rings: inputs split sync/scalar, outputs on scalar; weight
    slab wqkg[0] issued before xq so the first matmul starts ~5us not 24us
  * CH=16 chunking (4 chunks): fewer group boundaries, fewer DVE adds
  * qkT freed after the last S phase; stage-E wv slabs prefetched into
    the hole during the last Z phase
  (fp16 would cut quantization noise 8x at the same PE rate, but fp16
  matmuls fault the PE on this hw: NRT_EXEC_UNIT_UNRECOVERABLE)

Per-core dataflow (S_q = 1024 query rows, everything transposed so the
PE never needs an explicit transpose):
    B: qkT[d,s]  = sum_i W_qk[i,d] xT_blk[i,s]  -> SBUF resident
    C: ST[t,s]   = sum_d xT[d,t] qkT[d,s]       (per t-tile of 128 keys)
       PT[t,s]   = exp(ST * 1/sqrt(D))          -> SBUF chunk (fp16)
       rsacc[s] += PT (DVE, fp32)
    D: ZT[i,s]  += sum_t x[t,i] PT[t,s]         (chunked over t, fp32 acc)
    E: O[s,j]    = sum_i ZT[i,s] wvT[i,j] * (1/rowsum[s])
"""

import sys

sys.path.insert(0, "/opt/trn_rl_repo")

import ml_dtypes
import numpy as np

import concourse.bass as bass
from concourse import bacc
import concourse.mybir as mybir
import concourse.tile as tile
from concourse.bass_utils import run_bass_kernel_spmd

S = 8192          # sequence length
D = 2048          # model dim
NCORES = 8
SQ = S // NCORES  # 1024 query rows per core
P = 128           # partitions

ND = D // P       # 16 d-tiles (post-Wqk dim)
NI = D // P       # 16 i-tiles (input dim)
NT = S // P       # 64 key tiles
NSQ = SQ // P     # 8 query tiles per core
CH = 16           # key tiles per chunk
NCH = NT // CH    # 4 chunks
NLB = D // 512    # 4 column blocks of 512
SCALE = 1.0 / float(np.sqrt(np.float32(D)))

F32 = mybir.dt.float32
F16 = mybir.dt.bfloat16     # fp16 matmuls fault the PE (NRT_EXEC_UNIT_UNRECOVERABLE)
NPF16 = np.dtype(ml_dtypes.bfloat16)
AFT = mybir.ActivationFunctionType


def build_nc() -> bass.Bass:
    nc = bacc.Bacc()

    # [p, i, s] : xq[p, i, s] = x[core*SQ + s, i*128 + p]            (per-core)
    xq_p = nc.declare_dram_parameter("xq", [P, NI, SQ], F16, isOutput=False)
    # [t, p, d*128+f] : xt[t][p, d, f] = x[t*128 + f, d*128 + p]     (shared)
    xt_p = nc.declare_dram_parameter("xt", [NT, P, D], F16, isOutput=False)
    # [i, p, t*128+f] : xc[i][p, t, f] = x[t*128 + p, i*128 + f]     (shared)
    xc_p = nc.declare_dram_parameter("xc", [NI, P, NT * P], F16, isOutput=False)
    # [d, p, i, f] : wqkg[d][p, i, f] = W_qk[i*128+p, d*128+f], where
    # W_qk = wq.T @ wk is folded on the host (weights-only constant)
    wqkg_p = nc.declare_dram_parameter("wqkg", [ND, P, NI, P], F16, isOutput=False)
    # [jb, p, i, f] : wvt[jb][p, i, f] = wv[jb*512 + f, i*128 + p]   (shared)
    wvt_p = nc.declare_dram_parameter("wvt", [NLB, P, NI, 512], F16, isOutput=False)

    out_p = nc.declare_dram_parameter("out", [SQ, D], F32, isOutput=True)

    rs_d = nc.dram_tensor("rowsum_scratch", [SQ], F32)

    r1 = lambda ap: ap.rearrange("p (a f) -> p a f", a=1)

    with tile.TileContext(nc) as tc:
        # ---- small persistent pool (live across all stages) ----
        with tc.tile_pool(name="persist", bufs=1) as persist, \
             tc.tile_pool(name="z", bufs=1) as zpool, \
             tc.tile_pool(name="c_pt", bufs=1) as c_pt:
            ones = persist.tile([P, 1], F16, tag="ones")
            recip = persist.tile([P, NSQ], F32, tag="recip")
            rsacc = persist.tile([P, SQ], F32, tag="rsacc")
            rs16 = persist.tile([P, SQ], F16, tag="rs16")
            nc.vector.memset(ones, 1.0)
            nc.vector.memset(rsacc, 0.0)

            zacc = zpool.tile([P, NI, SQ], F32, tag="zacc")         # 64KB/part
            pT = c_pt.tile([P, CH, SQ], F16, tag="pt")              # 32KB/part

            # qkT in its own pool so it can be freed after the last S phase
            qk_cm = tc.tile_pool(name="qk", bufs=1)
            qk = qk_cm.__enter__()
            qkt = qk.tile([P, ND, SQ], F16, tag="qkt")              # 32KB/part

            # c_xt opens before stage B so the first xt tiles can be
            # prefetched at the head of the DMA queue (the later xt
            # dma_starts are gated by stage B's weight-slab pipeline)
            cxt_cm = tc.tile_pool(name="c_xt", bufs=3)
            c_xt = cxt_cm.__enter__()

            # ================= Stage B: qkT = W_qk.T @ xT_blk ============
            with tc.tile_pool(name="b_xq", bufs=1) as b_xq, \
                 tc.tile_pool(name="b_w", bufs=3) as b_w, \
                 tc.tile_pool(name="b_ps", bufs=4, space="PSUM") as b_ps:
                xq_sb = b_xq.tile([P, NI, SQ], F16, tag="xq")       # 32KB/part
                # weight slab for d=0 FIRST (split across both HWDGE rings)
                # so the PE can start early; xq tiles alternate rings, with
                # the d=1,2 slabs inserted by need time (~17us / ~24us)
                wqk_first = b_w.tile([P, NI, P], F16, tag="wqks")
                nc.sync.dma_start(out=wqk_first[:, 0:NI // 2, :],
                                  in_=wqkg_p[0][:, 0:NI // 2, :])
                nc.scalar.dma_start(out=wqk_first[:, NI // 2:, :],
                                    in_=wqkg_p[0][:, NI // 2:, :])
                wqk_pre = {0: wqk_first}

                def issue_xq(i, h):
                    # two HWDGE rings (gpsimd SWDGE as a third path measured
                    # 7us slower); h0/h1 halves issued separately so the
                    # first d-sweep's h0 group only waits on the h0 columns
                    # it actually reads — 1.5MB/ring critical path, not 2.25
                    eng = nc.sync if i % 2 == 0 else nc.scalar
                    hs = slice(h * 512, (h + 1) * 512)
                    eng.dma_start(out=xq_sb[:, i, hs], in_=xq_p[:, i, hs])

                def issue_wqk(d, eng):
                    sl = b_w.tile([P, NI, P], F16, tag="wqks")
                    eng.dma_start(out=sl, in_=wqkg_p[d])
                    wqk_pre[d] = sl

                for i in (0, 1, 2, 3):
                    issue_xq(i, 0)
                issue_wqk(1, nc.scalar)
                for i in (4, 5, 6, 7):
                    issue_xq(i, 0)
                issue_wqk(2, nc.sync)
                for i in range(8, NI):
                    issue_xq(i, 0)
                for i in range(NI):
                    issue_xq(i, 1)
                xts_pre = []
                for t in range(2):
                    xts = c_xt.tile([P, D], F16, tag="xts")
                    nc.sync.dma_start(out=xts, in_=xt_p[t])
                    xts_pre.append(xts)

                for d in range(ND):
                    if d in wqk_pre:
                        wqk_sl = wqk_pre[d]
                    else:
                        wqk_sl = b_w.tile([P, NI, P], F16, tag="wqks")
                        eng = nc.sync if d % 2 == 0 else nc.scalar
                        eng.dma_start(out=wqk_sl, in_=wqkg_p[d])
                    # NOTE: interleaving the two half-groups per i (to match
                    # the first d-sweep's consumption to DMA delivery rate)
                    # faults the PE at runtime (NRT_EXEC_UNIT_UNRECOVERABLE)
                    # despite passing CoreSim — keep the groups sequential.
                    for h in range(2):
                        hs = slice(h * 512, (h + 1) * 512)
                        bps = b_ps.tile([P, 512], F32, tag="bps")
                        for i in range(NI):
                            nc.tensor.matmul(
                                bps,
                                wqk_sl[:, i, :],
                                xq_sb[:, i, hs],
                                start=(i == 0),
                                stop=(i == NI - 1),
                            )
                        nc.scalar.copy(r1(qkt[:, d, hs]), r1(bps))

            # ============ Stages C+D: scores, exp, rowsum, Z =============
            def s_phase(ch, c_xt, c_sps):
                for tl in range(CH):
                    t = ch * CH + tl
                    if ch == 0 and tl < len(xts_pre):
                        xts = xts_pre[tl]
                    else:
                        xts = c_xt.tile([P, D], F16, tag="xts")
                        nc.sync.dma_start(out=xts, in_=xt_p[t])
                    for h in range(2):
                        hs = slice(h * 512, (h + 1) * 512)
                        sps = c_sps.tile([P, 512], F32, tag="sps")
                        for d in range(ND):
                            nc.tensor.matmul(
                                sps,
                                xts[:, d * P:(d + 1) * P],
                                qkt[:, d, hs],
                                start=(d == 0),
                                stop=(d == ND - 1),
                            )
                        nc.scalar.activation(
                            pT[:, tl, hs], sps, AFT.Exp, scale=SCALE
                        )
                        nc.vector.tensor_add(
                            rsacc[:, hs], rsacc[:, hs], pT[:, tl, hs]
                        )

            def z_tile(ch, i, c_xc, c_zps, zb):
                last_ch = ch == NCH - 1
                xcs = c_xc.tile([P, CH * P], F16, tag="xcs")
                nc.scalar.dma_start(
                    out=xcs,
                    in_=xc_p[i, :, ch * CH * P:(ch + 1) * CH * P],
                )
                for h in range(2):
                    hs = slice(h * 512, (h + 1) * 512)
                    zps = c_zps.tile([P, 512], F32, tag="zps")
                    for tl in range(CH):
                        nc.tensor.matmul(
                            zps,
                            xcs[:, tl * P:(tl + 1) * P],
                            pT[:, tl, hs],
                            start=(tl == 0),
                            stop=(tl == CH - 1),
                        )
                    if ch == 0:
                        nc.scalar.copy(r1(zacc[:, i, hs]), r1(zps))
                    elif not last_ch:
                        nc.vector.tensor_add(
                            zacc[:, i, hs], zacc[:, i, hs], zps
                        )
                    else:
                        nc.vector.tensor_add(zb[:, i, hs], zacc[:, i, hs], zps)

            with tc.tile_pool(name="c_xc", bufs=2) as c_xc, \
                 tc.tile_pool(name="c_sps", bufs=3, space="PSUM") as c_sps, \
                 tc.tile_pool(name="c_zps", bufs=3, space="PSUM") as c_zps:
                for ch in range(NCH - 1):
                    s_phase(ch, c_xt, c_sps)
                    for i in range(NI):
                        z_tile(ch, i, c_xc, c_zps, None)
                # last chunk's scores (qkT's final consumer)
                s_phase(NCH - 1, c_xt, c_sps)

            # free qkT; stage-E staging goes into the hole
            cxt_cm.__exit__(None, None, None)
            qk_cm.__exit__(None, None, None)

            with tc.tile_pool(name="zb", bufs=1) as zbp, \
                 tc.tile_pool(name="e_w", bufs=2) as e_w, \
                 tc.tile_pool(name="c2_misc", bufs=1) as c2_misc, \
                 tc.tile_pool(name="c2_xc", bufs=2) as c2_xc, \
                 tc.tile_pool(name="c2_zps", bufs=3, space="PSUM") as c2_zps, \
                 tc.tile_pool(name="c2_rs", bufs=1, space="PSUM") as c2_rs:
                zb = zbp.tile([P, NI, SQ], F16, tag="zb")           # 32KB/part
                # prefetch stage E's first wv slabs behind the last Z phase
                wv_pre = []
                for jb in range(2):
                    wv_sl = e_w.tile([P, NI, 512], F16, tag="wvsl")
                    nc.sync.dma_start(out=wv_sl, in_=wvt_p[jb])
                    wv_pre.append(wv_sl)

                # ---- last chunk's Z phase + rowsum finalize ----
                for i in range(NI):
                    z_tile(NCH - 1, i, c2_xc, c2_zps, zb)
                    if i == 2:
                        # rsacc is complete; the PE's two tiny matmuls slot
                        # in here while the DRAM bounce and reciprocal run
                        # under the remaining Z phase
                        nc.scalar.copy(r1(rs16), r1(rsacc))
                        rs_ps = c2_rs.tile([1, SQ], F32, tag="rsps")
                        for h in range(2):
                            hs = slice(h * 512, (h + 1) * 512)
                            nc.tensor.matmul(
                                rs_ps[0:1, hs], ones, rs16[:, hs],
                                start=True, stop=True,
                            )
                        rs_sb = c2_misc.tile([1, SQ], F32, tag="rssb")
                        nc.scalar.copy(rs_sb, rs_ps)
                        nc.sync.dma_start(out=rs_d[:], in_=rs_sb)
                        rs_t = c2_misc.tile([P, NSQ], F32, tag="rst")
                        nc.sync.dma_start(
                            out=rs_t, in_=rs_d[:].rearrange("(q p) -> p q", p=P)
                        )
                        nc.vector.reciprocal(recip, rs_t)

                # ============ Stage E: O = ZT.T @ wvT * recip ============
                with tc.tile_pool(name="e_o", bufs=3) as e_o, \
                     tc.tile_pool(name="e_ps", bufs=3, space="PSUM") as e_ps:
                    for jb in range(NLB):
                        if jb < 2:
                            wv_sl = wv_pre[jb]
                        else:
                            wv_sl = e_w.tile([P, NI, 512], F16, tag="wvsl")
                            nc.sync.dma_start(out=wv_sl, in_=wvt_p[jb])
                        for sq in range(NSQ):
                            ops = e_ps.tile([P, 512], F32, tag="ops")
                            for i in range(NI):
                                nc.tensor.matmul(
                                    ops,
                                    zb[:, i, sq * P:(sq + 1) * P],
                                    wv_sl[:, i, :],
                                    start=(i == 0),
                                    stop=(i == NI - 1),
                                )
                            osb = e_o.tile([P, 512], F32, tag="osb")
                            nc.scalar.activation(
                                osb, ops, AFT.Copy, scale=recip[:, sq:sq + 1]
                            )
                            rows = slice(sq * P, (sq + 1) * P)
                            if jb == NLB - 1 and sq == NSQ - 1:
                                # split the final tile across both rings so
                                # the last-output completion latency halves
                                nc.scalar.dma_start(
                                    out=out_p[rows, jb * 512:jb * 512 + 256],
                                    in_=osb[:, 0:256],
                                )
                                nc.sync.dma_start(
                                    out=out_p[rows, jb * 512 + 256:(jb + 1) * 512],
                                    in_=osb[:, 256:512],
                                )
                            else:
                                nc.scalar.dma_start(
                                    out=out_p[rows, jb * 512:(jb + 1) * 512],
                                    in_=osb,
                                )
    nc.finalize()
    return nc


def prep_inputs(token_encoding, w_q, w_k, w_v):
    """Host-side relayouts (to fp16) so every device DMA is wide/contiguous."""
    x = np.asarray(token_encoding, dtype=np.float32).astype(NPF16)
    wv = np.asarray(w_v, dtype=np.float32).astype(NPF16)

    x4 = x.reshape(NT, P, NI, P)
    # xt[t, p, d*128+f] = x[t*128+f, d*128+p]
    xt = np.ascontiguousarray(x4.transpose(0, 3, 2, 1)).reshape(NT, P, D)
    # xc[i, p, t*128+f] = x[t*128+p, i*128+f]
    xc = np.ascontiguousarray(x4.transpose(2, 1, 0, 3)).reshape(NI, P, NT * P)
    # fold the weight-only constant W_qk = wq.T @ wk (fp32), relayout to
    # column-slabs wqkg[d, p, i, f] = W_qk[i*128+p, d*128+f]
    wqk = (np.asarray(w_q, dtype=np.float32).T
           @ np.asarray(w_k, dtype=np.float32)).astype(NPF16)
    wqkg = np.ascontiguousarray(
        wqk.reshape(NI, P, ND, P).transpose(2, 1, 0, 3))
    # wvt[jb, p, i, f] = wv[jb*512+f, i*128+p]
    wvt = np.ascontiguousarray(wv.reshape(NLB, 512, NI, P).transpose(0, 3, 2, 1))

    in_maps = []
    for c in range(NCORES):
        xblk = x[c * SQ:(c + 1) * SQ]                # [1024, 2048]
        # xq[p, i, s] = x[c*SQ+s, i*128+p]
        xq = np.ascontiguousarray(xblk.reshape(SQ, NI, P).transpose(2, 1, 0))
        in_maps.append(
            {"xq": xq, "xt": xt, "xc": xc, "wqkg": wqkg, "wvt": wvt}
        )
    return in_maps


_NC_CACHE = None


def _get_nc():
    global _NC_CACHE
    if _NC_CACHE is None:
        _NC_CACHE = build_nc()
    return _NC_CACHE


def run(inputs: dict, trace: bool = False):
    in_maps = prep_inputs(**inputs)
    nc = _get_nc()
    res = run_bass_kernel_spmd(nc, in_maps, list(range(NCORES)), trace=trace)
    out = np.concatenate([res.results[c]["out"] for c in range(NCORES)], axis=0)
    return out, res


def kernel(**inputs) -> np.ndarray:
    out, _ = run(inputs, trace=False)
    return out


# revision 14
# speedup vs baseline: 1.0064x; 1.0064x over previous
"""Masked self-attention (mask is a no-op) on 8 Trainium2 NeuronCores — v2.

Math (reference):
    q = x @ wq.T ; k = x @ wk.T ; v = x @ wv.T
    O = softmax(q @ k.T / sqrt(D)) @ v

Factorized form (identical math up to fp reassociation):
    W_qk = wq.T @ wk                  # [D, D] weights-only, folded on host
    S    = (x_blk @ W_qk) @ x.T       # block of q @ k.T (unscaled)
    P    = exp(S / sqrt(D))           # unnormalized softmax
    O    = (P @ x) @ wv.T / rowsum(P)

Changes vs v1 (1216.7us):
  * rowsum off the PE: DVE accumulates sum_t PT[t,:] into fp32 rsacc;
    a single ones-matmul pair at the end replaces 128 rowsum matmuls
    (-64K PE cycles = -27us)
  * S/Z PSUM tiles are [P,512] halves with bufs=3: exp/copy never blocks
    the next matmul group (kills the ~200-700ns per-tile PE gaps)
  * two HWDGE rings: inputs split sync/scalar, outputs on scalar; weight
    slab wqkg[0] issued before xq so the first matmul starts ~5us not 24us
  * CH=16 chunking (4 chunks): fewer group boundaries, fewer DVE adds
  * qkT freed after the last S phase; stage-E wv slabs prefetched into
    the hole during the last Z phase
  (fp16 would cut quantization noise 8x at the same PE rate, but fp16
  matmuls fault the PE on this hw: NRT_EXEC_UNIT_UNRECOVERABLE)

Per-core dataflow (S_q = 1024 query rows, everything transposed so the
PE never needs an explicit transpose):
    B: qkT[d,s]  = sum_i W_qk[i,d] xT_blk[i,s]  -> SBUF resident
    C: ST[t,s]   = sum_d xT[d,t] qkT[d,s]       (per t-tile of 128 keys)
       PT[t,s]   = exp(ST * 1/sqrt(D))          -> SBUF chunk (fp16)
       rsacc[s] += PT (DVE, fp32)
    D: ZT[i,s]  += sum_t x[t,i] PT[t,s]         (chunked over t, fp32 acc)
    E: O[s,j]    = sum_i ZT[i,s] wvT[i,j] * (1/rowsum[s])
"""

import sys

sys.path.insert(0, "/opt/trn_rl_repo")

import ml_dtypes
import numpy as np

import concourse.bass as bass
from concourse import bacc
import concourse.mybir as mybir
import concourse.tile as tile
from concourse.bass_utils import run_bass_kernel_spmd

S = 8192          # sequence length
D = 2048          # model dim
NCORES = 8
SQ = S // NCORES  # 1024 query rows per core
P = 128           # partitions

ND = D // P       # 16 d-tiles (post-Wqk dim)
NI = D // P       # 16 i-tiles (input dim)
NT = S // P       # 64 key tiles
NSQ = SQ // P     # 8 query tiles per core
CH = 16           # key tiles per chunk
NCH = NT // CH    # 4 chunks
NLB = D // 512    # 4 column blocks of 512
SCALE = 1.0 / float(np.sqrt(np.float32(D)))

F32 = mybir.dt.float32
F16 = mybir.dt.bfloat16     # fp16 matmuls fault the PE (NRT_EXEC_UNIT_UNRECOVERABLE)
NPF16 = np.dtype(ml_dtypes.bfloat16)
AFT = mybir.ActivationFunctionType


def build_nc() -> bass.Bass:
    nc = bacc.Bacc()

    # [p, i, s] : xq[p, i, s] = x[core*SQ + s, i*128 + p]            (per-core)
    xq_p = nc.declare_dram_parameter("xq", [P, NI, SQ], F16, isOutput=False)
    # [t, p, d*128+f] : xt[t][p, d, f] = x[t*128 + f, d*128 + p]     (shared)
    xt_p = nc.declare_dram_parameter("xt", [NT, P, D], F16, isOutput=False)
    # [i, p, t*128+f] : xc[i][p, t, f] = x[t*128 + p, i*128 + f]     (shared)
    xc_p = nc.declare_dram_parameter("xc", [NI, P, NT * P], F16, isOutput=False)
    # [d, p, i, f] : wqkg[d][p, i, f] = W_qk[i*128+p, d*128+f], where
    # W_qk = wq.T @ wk is folded on the host (weights-only constant)
    wqkg_p = nc.declare_dram_parameter("wqkg", [ND, P, NI, P], F16, isOutput=False)
    # [jb, p, i, f] : wvt[jb][p, i, f] = wv[jb*512 + f, i*128 + p]   (shared)
    wvt_p = nc.declare_dram_parameter("wvt", [NLB, P, NI, 512], F16, isOutput=False)

    out_p = nc.declare_dram_parameter("out", [SQ, D], F32, isOutput=True)

    rs_d = nc.dram_tensor("rowsum_scratch", [SQ], F32)

    r1 = lambda ap: ap.rearrange("p (a f) -> p a f", a=1)

    with tile.TileContext(nc) as tc:
        # ---- small persistent pool (live across all stages) ----
        with tc.tile_pool(name="persist", bufs=1) as persist, \
             tc.tile_pool(name="z", bufs=1) as zpool, \
             tc.tile_pool(name="c_pt", bufs=1) as c_pt:
            ones = persist.tile([P, 1], F16, tag="ones")
            recip = persist.tile([P, NSQ], F32, tag="recip")
            rsacc = persist.tile([P, SQ], F32, tag="rsacc")
            rs16 = persist.tile([P, SQ], F16, tag="rs16")
            nc.vector.memset(ones, 1.0)
            nc.vector.memset(rsacc, 0.0)

            zacc = zpool.tile([P, NI, SQ], F32, tag="zacc")         # 64KB/part
            pT = c_pt.tile([P, CH, SQ], F16, tag="pt")              # 32KB/part

            # qkT in its own pool so it can be freed after the last S phase
            qk_cm = tc.tile_pool(name="qk", bufs=1)
            qk = qk_cm.__enter__()
            qkt = qk.tile([P, ND, SQ], F16, tag="qkt")              # 32KB/part

            # c_xt opens before stage B so the first xt tiles can be
            # prefetched at the head of the DMA queue (the later xt
            # dma_starts are gated by stage B's weight-slab pipeline)
            cxt_cm = tc.tile_pool(name="c_xt", bufs=3)
            c_xt = cxt_cm.__enter__()

            # ================= Stage B: qkT = W_qk.T @ xT_blk ============
            with tc.tile_pool(name="b_xq", bufs=1) as b_xq, \
                 tc.tile_pool(name="b_w", bufs=3) as b_w, \
                 tc.tile_pool(name="b_ps", bufs=4, space="PSUM") as b_ps:
                xq_sb = b_xq.tile([P, NI, SQ], F16, tag="xq")       # 32KB/part
                # weight slab for d=0 FIRST (split across both HWDGE rings)
                # so the PE can start early; xq tiles alternate rings, with
                # the d=1,2 slabs inserted by need time (~17us / ~24us)
                wqk_first = b_w.tile([P, NI, P], F16, tag="wqks")
                nc.sync.dma_start(out=wqk_first[:, 0:NI // 2, :],
                                  in_=wqkg_p[0][:, 0:NI // 2, :])
                nc.scalar.dma_start(out=wqk_first[:, NI // 2:, :],
                                    in_=wqkg_p[0][:, NI // 2:, :])
                wqk_pre = {0: wqk_first}

                def issue_xq(i):
                    # two HWDGE rings; adding the gpsimd SWDGE ring as a
                    # third path measured 7us SLOWER (descriptor-gen cost)
                    eng = nc.sync if i % 2 == 0 else nc.scalar
                    eng.dma_start(out=xq_sb[:, i, :], in_=xq_p[:, i, :])

                def issue_wqk(d, eng):
                    sl = b_w.tile([P, NI, P], F16, tag="wqks")
                    eng.dma_start(out=sl, in_=wqkg_p[d])
                    wqk_pre[d] = sl

                for i in (0, 1, 2, 3):
                    issue_xq(i)
                issue_wqk(1, nc.scalar)
                for i in (4, 5, 6, 7):
                    issue_xq(i)
                issue_wqk(2, nc.sync)
                for i in range(8, NI):
                    issue_xq(i)
                xts_pre = []
                for t in range(2):
                    xts = c_xt.tile([P, D], F16, tag="xts")
                    nc.sync.dma_start(out=xts, in_=xt_p[t])
                    xts_pre.append(xts)

                for d in range(ND):
                    if d in wqk_pre:
                        wqk_sl = wqk_pre[d]
                    else:
                        wqk_sl = b_w.tile([P, NI, P], F16, tag="wqks")
                        eng = nc.sync if d % 2 == 0 else nc.scalar
                        eng.dma_start(out=wqk_sl, in_=wqkg_p[d])
                    # NOTE: interleaving the two half-groups per i (to match
                    # the first d-sweep's consumption to DMA delivery rate)
                    # faults the PE at runtime (NRT_EXEC_UNIT_UNRECOVERABLE)
                    # despite passing CoreSim — keep the groups sequential.
                    for h in range(2):
                        hs = slice(h * 512, (h + 1) * 512)
                        bps = b_ps.tile([P, 512], F32, tag="bps")
                        for i in range(NI):
                            nc.tensor.matmul(
                                bps,
                                wqk_sl[:, i, :],
                                xq_sb[:, i, hs],
                                start=(i == 0),
                                stop=(i == NI - 1),
                            )
                        nc.scalar.copy(r1(qkt[:, d, hs]), r1(bps))

            # ============ Stages C+D: scores, exp, rowsum, Z =============
            def s_phase(ch, c_xt, c_sps):
                for tl in range(CH):
                    t = ch * CH + tl
                    if ch == 0 and tl < len(xts_pre):
                        xts = xts_pre[tl]
                    else:
                        xts = c_xt.tile([P, D], F16, tag="xts")
                        nc.sync.dma_start(out=xts, in_=xt_p[t])
                    for h in range(2):
                        hs = slice(h * 512, (h + 1) * 512)
                        sps = c_sps.tile([P, 512], F32, tag="sps")
                        for d in range(ND):
                            nc.tensor.matmul(
                                sps,
                                xts[:, d * P:(d + 1) * P],
                                qkt[:, d, hs],
                                start=(d == 0),
                                stop=(d == ND - 1),
                            )
                        nc.scalar.activation(
                            pT[:, tl, hs], sps, AFT.Exp, scale=SCALE
                        )
                        nc.vector.tensor_add(
                            rsacc[:, hs], rsacc[:, hs], pT[:, tl, hs]
                        )

            def z_tile(ch, i, c_xc, c_zps, zb):
                last_ch = ch == NCH - 1
                xcs = c_xc.tile([P, CH * P], F16, tag="xcs")
                nc.scalar.dma_start(
                    out=xcs,
                    in_=xc_p[i, :, ch * CH * P:(ch + 1) * CH * P],
                )
                for h in range(2):
                    hs = slice(h * 512, (h + 1) * 512)
                    zps = c_zps.tile([P, 512], F32, tag="zps")
                    for tl in range(CH):
                        nc.tensor.matmul(
                            zps,
                            xcs[:, tl * P:(tl + 1) * P],
                            pT[:, tl, hs],
                            start=(tl == 0),
                            stop=(tl == CH - 1),
                        )
                    if ch == 0:
                        nc.scalar.copy(r1(zacc[:, i, hs]), r1(zps))
                    elif not last_ch:
                        nc.vector.tensor_add(
                            zacc[:, i, hs], zacc[:, i, hs], zps
                        )
                    else:
                        nc.vector.tensor_add(zb[:, i, hs], zacc[:, i, hs], zps)

            with tc.tile_pool(name="c_xc", bufs=2) as c_xc, \
                 tc.tile_pool(name="c_sps", bufs=3, space="PSUM") as c_sps, \
                 tc.tile_pool(name="c_zps", bufs=3, space="PSUM") as c_zps:
                for ch in range(NCH - 1):
                    s_phase(ch, c_xt, c_sps)
                    for i in range(NI):
                        z_tile(ch, i, c_xc, c_zps, None)
                # last chunk's scores (qkT's final consumer)
                s_phase(NCH - 1, c_xt, c_sps)

            # free qkT; stage-E staging goes into the hole
            cxt_cm.__exit__(None, None, None)
            qk_cm.__exit__(None, None, None)

            with tc.tile_pool(name="zb", bufs=1) as zbp, \
                 tc.tile_pool(name="e_w", bufs=2) as e_w, \
                 tc.tile_pool(name="c2_misc", bufs=1) as c2_misc, \
                 tc.tile_pool(name="c2_xc", bufs=2) as c2_xc, \
                 tc.tile_pool(name="c2_zps", bufs=3, space="PSUM") as c2_zps, \
                 tc.tile_pool(name="c2_rs", bufs=1, space="PSUM") as c2_rs:
                zb = zbp.tile([P, NI, SQ], F16, tag="zb")           # 32KB/part
                # prefetch stage E's first wv slabs behind the last Z phase
                wv_pre = []
                for jb in range(2):
                    wv_sl = e_w.tile([P, NI, 512], F16, tag="wvsl")
                    nc.sync.dma_start(out=wv_sl, in_=wvt_p[jb])
                    wv_pre.append(wv_sl)

                # ---- last chunk's Z phase + rowsum finalize ----
                for i in range(NI):
                    z_tile(NCH - 1, i, c2_xc, c2_zps, zb)
                    if i == 2:
                        # rsacc is complete; the PE's two tiny matmuls slot
                        # in here while the DRAM bounce and reciprocal run
                        # under the remaining Z phase
                        nc.scalar.copy(r1(rs16), r1(rsacc))
                        rs_ps = c2_rs.tile([1, SQ], F32, tag="rsps")
                        for h in range(2):
                            hs = slice(h * 512, (h + 1) * 512)
                            nc.tensor.matmul(
                                rs_ps[0:1, hs], ones, rs16[:, hs],
                                start=True, stop=True,
                            )
                        rs_sb = c2_misc.tile([1, SQ], F32, tag="rssb")
                        nc.scalar.copy(rs_sb, rs_ps)
                        nc.sync.dma_start(out=rs_d[:], in_=rs_sb)
                        rs_t = c2_misc.tile([P, NSQ], F32, tag="rst")
                        nc.sync.dma_start(
                            out=rs_t, in_=rs_d[:].rearrange("(q p) -> p q", p=P)
                        )
                        nc.vector.reciprocal(recip, rs_t)

                # ============ Stage E: O = ZT.T @ wvT * recip ============
                with tc.tile_pool(name="e_o", bufs=3) as e_o, \
                     tc.tile_pool(name="e_ps", bufs=3, space="PSUM") as e_ps:
                    for jb in range(NLB):
                        if jb < 2:
                            wv_sl = wv_pre[jb]
                        else:
                            wv_sl = e_w.tile([P, NI, 512], F16, tag="wvsl")
                            nc.sync.dma_start(out=wv_sl, in_=wvt_p[jb])
                        for sq in range(NSQ):
                            ops = e_ps.tile([P, 512], F32, tag="ops")
                            for i in range(NI):
                                nc.tensor.matmul(
                                    ops,
                                    zb[:, i, sq * P:(sq + 1) * P],
                                    wv_sl[:, i, :],
                                    start=(i == 0),
                                    stop=(i == NI - 1),
                                )
                            osb = e_o.tile([P, 512], F32, tag="osb")
                            nc.scalar.activation(
                                osb, ops, AFT.Copy, scale=recip[:, sq:sq + 1]
                            )
                            rows = slice(sq * P, (sq + 1) * P)
                            if jb == NLB - 1 and sq == NSQ - 1:
                                # split the final tile across both rings so
                                # the last-output completion latency halves
                                nc.scalar.dma_start(
                                    out=out_p[rows, jb * 512:jb * 512 + 256],
                                    in_=osb[:, 0:256],
                                )
                                nc.sync.dma_start(
                                    out=out_p[rows, jb * 512 + 256:(jb + 1) * 512],
                                    in_=osb[:, 256:512],
                                )
                            else:
                                nc.scalar.dma_start(
                                    out=out_p[rows, jb * 512:(jb + 1) * 512],
                                    in_=osb,
                                )
    nc.finalize()
    return nc


def prep_inputs(token_encoding, w_q, w_k, w_v):
    """Host-side relayouts (to fp16) so every device DMA is wide/contiguous."""
    x = np.asarray(token_encoding, dtype=np.float32).astype(NPF16)
    wv = np.asarray(w_v, dtype=np.float32).astype(NPF16)

    x4 = x.reshape(NT, P, NI, P)
    # xt[t, p, d*128+f] = x[t*128+f, d*128+p]
    xt = np.ascontiguousarray(x4.transpose(0, 3, 2, 1)).reshape(NT, P, D)
    # xc[i, p, t*128+f] = x[t*128+p, i*128+f]
    xc = np.ascontiguousarray(x4.transpose(2, 1, 0, 3)).reshape(NI, P, NT * P)
    # fold the weight-only constant W_qk = wq.T @ wk (fp32), relayout to
    # column-slabs wqkg[d, p, i, f] = W_qk[i*128+p, d*128+f]
    wqk = (np.asarray(w_q, dtype=np.float32).T
           @ np.asarray(w_k, dtype=np.float32)).astype(NPF16)
    wqkg = np.ascontiguousarray(
        wqk.reshape(NI, P, ND, P).transpose(2, 1, 0, 3))
    # wvt[jb, p, i, f] = wv[jb*512+f, i*128+p]
    wvt = np.ascontiguousarray(wv.reshape(NLB, 512, NI, P).transpose(0, 3, 2, 1))

    in_maps = []
    for c in range(NCORES):
        xblk = x[c * SQ:(c + 1) * SQ]                # [1024, 2048]
        # xq[p, i, s] = x[c*SQ+s, i*128+p]
        xq = np.ascontiguousarray(xblk.reshape(SQ, NI, P).transpose(2, 1, 0))
        in_maps.append(
            {"xq": xq, "xt": xt, "xc": xc, "wqkg": wqkg, "wvt": wvt}
        )
    return in_maps


_NC_CACHE = None


def _get_nc():
    global _NC_CACHE
    if _NC_CACHE is None:
        _NC_CACHE = build_nc()
    return _NC_CACHE


def run(inputs: dict, trace: bool = False):
    in_maps = prep_inputs(**inputs)
    nc = _get_nc()
    res = run_bass_kernel_spmd(nc, in_maps, list(range(NCORES)), trace=trace)
    out = np.concatenate([res.results[c]["out"] for c in range(NCORES)], axis=0)
    return out, res


def kernel(**inputs) -> np.ndarray:
    out, _ = run(inputs, trace=False)
    return out
